# revision 24
# baseline (speedup 1.0000x reference)
"""Causal self-attention (B=2, T=2048, D=1024, H=16, Dh=64) on 8 TRN2 cores.

Sharding: core c = 4*b + g -> batch b (data parallel), head group g of 4
heads (tensor parallel on heads for Wq/Wk/Wv, column-split of the proj
input with the resulting partial-sum reduction done host-side at unshard).

Per-core dataflow (layouts chosen so no on-device transposes are needed):
  qT,kT [256, 2048] bf16 = W{q,k}_g @ x.T   (lhsT = W{q,k}_g.T from host)
  v     [t-block 128, 4 heads x (64 v | 64 ones)] bf16
  attention, transposed: PT[tk, tq] = kT_h-block.T @ qT_h (bf16), exp on
  ACT -> bf16, causal mask as post-exp 0/1 multiply on GPSIMD,
  AV: yT[d, tq] + softmax column sums free via the ones columns of v
  normalize: yT * recip(sums) -> ytsb [256, 2048] f32r (proj lhsT layout)
  proj partial: out[t, :] = ytsb.T-block @ Wp_gT  (f32r)
Host: out[b] = sum_g partial[4b+g] + bp.

Emission order interleaves projection work between attention segments to
keep the PE HAM-warm, and frees PSUM accumulators before the (expensive)
reciprocal so the vector engine never stalls the PE.
"""

import numpy as np

import concourse.bass as bass
import concourse.mybir as mybir
import concourse.tile as tile
from concourse import bacc
from concourse import bass_utils

F32 = mybir.dt.float32
F32R = mybir.dt.float32r
BF16 = mybir.dt.bfloat16

B, T, D = 2, 2048, 1024
H, DH = 16, 64
N_CORES = 8
HPC = 4            # heads per core
GD = HPC * DH      # 256 feature cols per core
KT = D // 128      # 8 k-tiles over the model dim
TB = T // 128      # 16 t-blocks of 128
SCL = 0.125        # logit scale 1/sqrt(Dh)

_cache = {}


def _build():
    nc = bacc.Bacc("TRN2", target_bir_lowering=False, debug=False,
                   num_devices=N_CORES)

    xT_d = nc.dram_tensor("xT", [D, T], BF16, kind="ExternalInput")
    wqT_d = nc.dram_tensor("wqT", [D, GD], BF16, kind="ExternalInput")
    wkT_d = nc.dram_tensor("wkT", [D, GD], BF16, kind="ExternalInput")
    wvT_d = nc.dram_tensor("wvT", [D, GD], BF16, kind="ExternalInput")
    wpT_d = nc.dram_tensor("wpT", [GD, D], F32R, kind="ExternalInput")
    bq_d = nc.dram_tensor("bq2", [128, 2], F32, kind="ExternalInput")
    bk_d = nc.dram_tensor("bk2", [128, 2], F32, kind="ExternalInput")
    bvb_d = nc.dram_tensor("bvb", [128, GD], F32, kind="ExternalInput")
    msk_d = nc.dram_tensor("mask01", [128, 128], BF16, kind="ExternalInput")
    out_d = nc.dram_tensor("out", [T, D], F32, kind="ExternalOutput")
    wrm_d = nc.dram_tensor("wrm", [128, 1], F32, kind="ExternalOutput")

    with tile.TileContext(nc) as tc:
        with (
            tc.tile_pool(name="const", bufs=1) as cp,
            tc.tile_pool(name="big", bufs=1) as bp_,
            tc.tile_pool(name="work", bufs=4) as wp_,
            tc.tile_pool(name="outp", bufs=6) as op_,
            tc.tile_pool(name="pA", bufs=2, space="PSUM") as pA,
            tc.tile_pool(name="pB", bufs=1, space="PSUM") as pB,
            tc.tile_pool(name="pC", bufs=2, space="PSUM") as pC,
        ):
            # ---- loads: wq/wk first (qk-proj is the first compute), x
            # spread over three DMA-capable queues ----
            wq = cp.tile([128, KT, GD], BF16, tag="wq", name="wq")
            wk = cp.tile([128, KT, GD], BF16, tag="wk", name="wk")
            wv = cp.tile([128, KT, GD], BF16, tag="wv", name="wv")
            nc.scalar.dma_start(wq[:], wqT_d.rearrange("(a p) m -> p a m", p=128))
            nc.sync.dma_start(wk[:], wkT_d.rearrange("(a p) m -> p a m", p=128))
            xt = []
            for k in range(KT):
                t_ = cp.tile([128, T], BF16, tag=f"xt{k}", name=f"xt{k}")
                eng = (nc.sync, nc.scalar, nc.gpsimd)[k % 3]
                eng.dma_start(t_[:], xT_d[k * 128:(k + 1) * 128, :])
                xt.append(t_)
            nc.gpsimd.dma_start(wv[:], wvT_d.rearrange("(a p) m -> p a m", p=128))
            bq2 = cp.tile([128, 2], F32, tag="bq2", name="bq2")
            bk2 = cp.tile([128, 2], F32, tag="bk2", name="bk2")
            bvb = cp.tile([128, GD], F32, tag="bvb", name="bvb")
            msk = cp.tile([128, 128], BF16, tag="msk", name="msk")
            nc.sync.dma_start(bq2[:], bq_d[:])
            nc.sync.dma_start(bk2[:], bk_d[:])
            nc.sync.dma_start(bvb[:], bvb_d[:])
            nc.sync.dma_start(msk[:], msk_d[:])
            wpt = []
            for p in range(2):
                t_ = cp.tile([128, D], F32R, tag=f"wp{p}", name=f"wp{p}")
                nc.scalar.dma_start(t_[:], wpT_d[p * 128:(p + 1) * 128, :])
                wpt.append(t_)

            qt = [bp_.tile([128, T], BF16, tag=f"qt{m}", name=f"qt{m}")
                  for m in range(2)]
            kt = [bp_.tile([128, T], BF16, tag=f"kt{m}", name=f"kt{m}")
                  for m in range(2)]
            ytsb = [bp_.tile([128, T], F32R, tag=f"yt{p}", name=f"yt{p}")
                    for p in range(2)]
            vt = [bp_.tile([128, 4, 2, DH], BF16, tag=f"v{t}", name=f"v{t}")
                  for t in range(TB)]

            def qk_group(dst, w, b2, m, n):
                ps = pC.tile([128, 512], F32, tag=pC.name, name="psqk")
                for k in range(KT):
                    nc.tensor.matmul(
                        ps[:],
                        w[:, k, m * 128:(m + 1) * 128],
                        xt[k][:, n * 512:(n + 1) * 512],
                        start=(k == 0), stop=(k == KT - 1),
                    )
                nc.vector.tensor_scalar_add(
                    dst[m][:, n * 512:(n + 1) * 512], ps[:], b2[:, m:m + 1],
                )

            def v_group(t):
                nc.gpsimd.memset(vt[t][:, :, 1, :], 1.0)
                ps = pC.tile([128, 512], F32, tag=pC.name, name="psv")
                for k in range(KT):
                    nc.tensor.matmul(
                        ps[:, 0:GD],
                        xt[k][:, t * 128:(t + 1) * 128],
                        wv[:, k, :],
                        start=(k == 0), stop=(k == KT - 1),
                    )
                nc.vector.tensor_add(
                    vt[t][:, :, 0, :],
                    ps[:, 0:GD].rearrange("p (h d) -> p h d", h=4),
                    bvb.rearrange("p (h d) -> p h d", h=4),
                )

            def proj_group(t, copy_eng):
                ob = op_.tile([128, 1024], F32, tag="ob", name="ob")
                for n in range(2):
                    po = pC.tile([128, 512], F32, tag=pC.name, name="pso")
                    for p in range(2):
                        nc.tensor.matmul(
                            po[:],
                            ytsb[p][:, 128 * t:128 * (t + 1)],
                            wpt[p][:, 512 * n:512 * (n + 1)],
                            start=(p == 0), stop=(p == 1),
                        )
                    if (copy_eng == "act") == (n == 0):
                        nc.scalar.copy(ob[:, 512 * n:512 * (n + 1)], po[:])
                    else:
                        nc.vector.tensor_copy(ob[:, 512 * n:512 * (n + 1)], po[:])
                eng = (nc.sync, nc.scalar)[t % 2]
                eng.dma_start(out_d[128 * t:128 * (t + 1), :], ob[:])

            def attention_seg(Ti, hp, j, fillers, every, last=False):
                h = 2 * hp + j
                ytp = pB.tile([128, 1024], F32, tag=pB.name, name="psyt")
                nblk = 8 * (Ti + 1)
                SKEW = 4       # AV trails QK/exp: the PE never
                pend = []      # waits on an exp that was just issued
                def do_av(tkb, ptsb):
                    s = max(0, 128 * tkb - 1024 * Ti)
                    for bk in range(2):
                        c0, c1 = max(s, 512 * bk), 512 * (bk + 1)
                        if c0 >= c1:
                            continue
                        nc.tensor.matmul(
                            ytp[:, c0:c1],
                            vt[tkb][:, h, :, :].rearrange("p a d -> p (a d)"),
                            ptsb[:, c0:c1],
                            start=(tkb == 0), stop=(tkb == nblk - 1),
                        )
                for tkb in range(nblk + SKEW):
                    if tkb < nblk:
                        s = max(0, 128 * tkb - 1024 * Ti)
                        pt = pA.tile([128, 1024], F32, tag=pA.name,
                                     name="pspt")
                        for bk in range(2):
                            c0, c1 = max(s, 512 * bk), 512 * (bk + 1)
                            if c0 >= c1:
                                continue
                            nc.tensor.matmul(
                                pt[:, c0:c1],
                                kt[hp][64 * j:64 * j + 64,
                                       128 * tkb:128 * (tkb + 1)],
                                qt[hp][64 * j:64 * j + 64,
                                       1024 * Ti + c0:1024 * Ti + c1],
                                start=True, stop=True,
                            )
                        ptsb = wp_.tile([128, 1024], BF16, tag="ptsb",
                                        name="ptsb", bufs=8)
                        nc.scalar.activation(
                            ptsb[:, s:1024], pt[:, s:1024],
                            mybir.ActivationFunctionType.Exp, scale=SCL,
                        )
                        if 128 * tkb >= 1024 * Ti:  # diagonal block
                            nc.gpsimd.tensor_mul(
                                ptsb[:, s:s + 128], ptsb[:, s:s + 128],
                                msk[:],
                            )
                        pend.append((tkb, ptsb))
                    if tkb >= SKEW:
                        do_av(*pend.pop(0))
                    if fillers and tkb % every == every - 1:
                        fillers.pop(0)()
                while pend:
                    do_av(*pend.pop(0))
                # free the PSUM accumulator promptly; 1/sums is computed as
                # exp(-ln(sums)) on ACT (same table set as the attention exp),
                # keeping the expensive reciprocal off the DVE
                if not last:
                    src = wp_.tile([128, 1024], F32, tag="ysb", name="ysb")
                    nc.vector.tensor_copy(src[:], ytp[:])
                else:
                    src = ytp
                rc = wp_.tile([64, 1024], F32, tag="recip", name="recip")
                for q in range(4):
                    cs = slice(256 * q, 256 * (q + 1))
                    nc.vector.reciprocal(rc[:, cs], src[64:128, cs])
                    nc.vector.tensor_mul(
                        ytsb[hp][64 * j:64 * j + 64,
                                 1024 * Ti + 256 * q:1024 * Ti + 256 * (q + 1)],
                        src[0:64, cs], rc[:, cs],
                    )

            # ---- schedule: only q/k m0 runs before attention; everything
            # else drips into attention segments as PE filler groups ----
            for n in range(4):
                qk_group(qt, wq, bq2, 0, n)
                qk_group(kt, wk, bk2, 0, n)

            f_v07 = [lambda t=t: v_group(t) for t in range(8)]
            f_qk1 = []
            for n in range(4):
                f_qk1.append(lambda n=n: qk_group(kt, wk, bk2, 1, n))
                f_qk1.append(lambda n=n: qk_group(qt, wq, bq2, 1, n))
            f_v = [lambda t=t: v_group(t) for t in range(8, 16)]
            f_p0 = [lambda t=t: proj_group(t, "act") for t in range(8)]

            # dummy matmuls whose only purpose is to keep the PE HAM-warm
            # through exp-paced stretches; each segment accumulates into one
            # scratch PSUM tile read once, chained into a debug output so
            # nothing dead-code-eliminates them
            wrm = bp_.tile([128, 1], F32, tag="wrm", name="wrm")
            def warm_seg(n):
                ps = pC.tile([128, 512], F32, tag=pC.name, name="pswrm")
                box = [0]
                def g():
                    nc.tensor.matmul(ps[:], msk[:], qt[0][:, 0:512],
                                     start=(box[0] == 0), stop=(box[0] == n - 1))
                    box[0] += 1
                def fin():
                    nc.vector.reduce_max(wrm[:], ps[:],
                                         axis=mybir.AxisListType.X)
                return [g] * (n - 1) + [lambda: (g(), fin())]

            def interleave(a, b):
                out = []
                for x, y in zip(a, b):
                    out += [x, y]
                return out

            attention_seg(0, 0, 0, f_v07, 1)
            attention_seg(0, 0, 1, f_qk1, 1)
            attention_seg(0, 1, 0, f_v, 1)
            attention_seg(0, 1, 1, warm_seg(8), 2)
            attention_seg(1, 0, 0, warm_seg(8), 2)
            attention_seg(1, 0, 1, f_p0[0:4], 3)
            attention_seg(1, 1, 0, f_p0[4:8], 3)
            attention_seg(1, 1, 1, warm_seg(8), 2, last=True)
            for t in range(8, 16):
                proj_group(t, "act")
            nc.sync.dma_start(wrm_d[:], wrm[:])

    nc.compile()
    return nc


def _shard(x, Wq, bq, Wk, bk, Wv, bv, Wp, bp):
    import ml_dtypes
    f32 = np.float32
    bf16 = ml_dtypes.bfloat16
    mask01 = np.triu(np.ones((128, 128), f32)).astype(bf16)
    in_maps = []
    for c in range(N_CORES):
        b, g = divmod(c, HPC)
        sl = slice(GD * g, GD * (g + 1))
        in_maps.append({
            "xT": np.ascontiguousarray(x[b].T).astype(bf16),
            "wqT": np.ascontiguousarray(Wq[sl, :].T).astype(bf16),
            "wkT": np.ascontiguousarray(Wk[sl, :].T).astype(bf16),
            "wvT": np.ascontiguousarray(Wv[sl, :].T).astype(bf16),
            "wpT": np.ascontiguousarray(Wp[:, sl].T, dtype=f32),
            "bq2": np.ascontiguousarray(bq[sl].reshape(2, 128).T, dtype=f32),
            "bk2": np.ascontiguousarray(bk[sl].reshape(2, 128).T, dtype=f32),
            "bvb": np.broadcast_to(bv[sl], (128, GD)).astype(f32),
            "mask01": mask01,
        })
    return in_maps


def run(inputs, trace=False):
    """Run the SPMD kernel; returns (output [B,T,D] f32, BassKernelResults)."""
    if "nc" not in _cache:
        _cache["nc"] = _build()
    nc = _cache["nc"]
    in_maps = _shard(**inputs)
    if trace:
        _install_ntff_hook()
    res = bass_utils.run_bass_kernel_spmd(
        nc, in_maps, core_ids=list(range(N_CORES)), trace=trace,
    )
    bp = np.asarray(inputs["bp"], dtype=np.float32)
    out = np.empty((B, T, D), dtype=np.float32)
    for b in range(B):
        acc = res.results[4 * b]["out"].astype(np.float32)
        for g in range(1, HPC):
            acc = acc + res.results[4 * b + g]["out"]
        out[b] = acc + bp
    return out, res


def kernel(**inputs):
    out, _ = run(inputs, trace=False)
    return out


def _install_ntff_hook():
    """antenv.axon_hooks is absent on this image; inject it so
    run_bass_kernel_spmd(trace=True) can capture NTFF profiles."""
    import sys, types
    if "antenv.axon_hooks" in sys.modules:
        return
    try:
        mod = types.ModuleType("antenv.axon_hooks")
        mod._hook = None
        mod.set_axon_ntff_profile_hook = lambda h: setattr(mod, "_hook", h)
        mod.get_axon_ntff_profile_hook = lambda: mod._hook
        sys.modules["antenv.axon_hooks"] = mod
        import antenv
        antenv.axon_hooks = mod
        from trn_agent_boot.trn_boot import _ntff_profile_via_ctypes
        mod.set_axon_ntff_profile_hook(
            _ntff_profile_via_ctypes("/opt/axon/libaxon_pjrt.so"))
    except Exception:
        pass


# revision 25
# speedup vs baseline: 1.1448x; 1.1448x over previous
"""Causal self-attention (B=2, T=2048, D=1024, H=16, Dh=64) on 8 TRN2 cores.

Sharding: core c = 4*b + g -> batch b (data parallel), head group g of 4
heads (tensor parallel on heads for Wq/Wk/Wv, column-split of the proj
input with the resulting partial-sum reduction done host-side at unshard).

Per-core dataflow (layouts chosen so no on-device transposes are needed):
  qT,kT [256, 2048] bf16 = W{q,k}_g @ x.T   (lhsT = W{q,k}_g.T from host)
  v     [t-block 128, 4 heads x (64 v | 64 ones)] bf16
  attention, transposed: PT[tk, tq] = kT_h-block.T @ qT_h (bf16), exp on
  ACT -> bf16, causal mask as post-exp 0/1 multiply on GPSIMD,
  AV: yT[d, tq] + softmax column sums free via the ones columns of v
  normalize: yT * recip(sums) -> ytsb [256, 2048] f32r (proj lhsT layout)
  proj partial: out[t, :] = ytsb.T-block @ Wp_gT  (f32r)
Host: out[b] = sum_g partial[4b+g] + bp.

Emission order interleaves projection work between attention segments to
keep the PE HAM-warm, and frees PSUM accumulators before the (expensive)
reciprocal so the vector engine never stalls the PE.
"""

import numpy as np

import concourse.bass as bass
import concourse.mybir as mybir
import concourse.tile as tile
from concourse import bacc
from concourse import bass_utils

F32 = mybir.dt.float32
F32R = mybir.dt.float32r
BF16 = mybir.dt.bfloat16

B, T, D = 2, 2048, 1024
H, DH = 16, 64
N_CORES = 8
HPC = 4            # heads per core
GD = HPC * DH      # 256 feature cols per core
KT = D // 128      # 8 k-tiles over the model dim
TB = T // 128      # 16 t-blocks of 128
SCL = 0.125        # logit scale 1/sqrt(Dh)

_cache = {}


def _build():
    nc = bacc.Bacc("TRN2", target_bir_lowering=False, debug=False,
                   num_devices=N_CORES)

    xT_d = nc.dram_tensor("xT", [D, T], BF16, kind="ExternalInput")
    wqT_d = nc.dram_tensor("wqT", [D, GD], BF16, kind="ExternalInput")
    wkT_d = nc.dram_tensor("wkT", [D, GD], BF16, kind="ExternalInput")
    wvT_d = nc.dram_tensor("wvT", [D, GD], BF16, kind="ExternalInput")
    wpT_d = nc.dram_tensor("wpT", [GD, D], F32R, kind="ExternalInput")
    bq_d = nc.dram_tensor("bq2", [128, 2], F32, kind="ExternalInput")
    bk_d = nc.dram_tensor("bk2", [128, 2], F32, kind="ExternalInput")
    bvb_d = nc.dram_tensor("bvb", [128, GD], F32, kind="ExternalInput")
    msk_d = nc.dram_tensor("mask01", [128, 128], BF16, kind="ExternalInput")
    out_d = nc.dram_tensor("out", [T, D], F32, kind="ExternalOutput")
    wrm_d = nc.dram_tensor("wrm", [128, 1], F32, kind="ExternalOutput")

    with tile.TileContext(nc) as tc:
        with (
            tc.tile_pool(name="const", bufs=1) as cp,
            tc.tile_pool(name="big", bufs=1) as bp_,
            tc.tile_pool(name="work", bufs=4) as wp_,
            tc.tile_pool(name="outp", bufs=6) as op_,
            tc.tile_pool(name="pA", bufs=2, space="PSUM") as pA,
            tc.tile_pool(name="pB", bufs=1, space="PSUM") as pB,
            tc.tile_pool(name="pC", bufs=2, space="PSUM") as pC,
        ):
            # ---- loads: wq/wk first (qk-proj is the first compute), x
            # spread over three DMA-capable queues ----
            wq = cp.tile([128, KT, GD], BF16, tag="wq", name="wq")
            wk = cp.tile([128, KT, GD], BF16, tag="wk", name="wk")
            wv = cp.tile([128, KT, GD], BF16, tag="wv", name="wv")
            nc.scalar.dma_start(wq[:], wqT_d.rearrange("(a p) m -> p a m", p=128))
            nc.sync.dma_start(wk[:], wkT_d.rearrange("(a p) m -> p a m", p=128))
            xt = []
            for k in range(KT):
                t_ = cp.tile([128, T], BF16, tag=f"xt{k}", name=f"xt{k}")
                eng = (nc.sync, nc.scalar, nc.gpsimd)[k % 3]
                eng.dma_start(t_[:], xT_d[k * 128:(k + 1) * 128, :])
                xt.append(t_)
            nc.gpsimd.dma_start(wv[:], wvT_d.rearrange("(a p) m -> p a m", p=128))
            bq2 = cp.tile([128, 2], F32, tag="bq2", name="bq2")
            bk2 = cp.tile([128, 2], F32, tag="bk2", name="bk2")
            bvb = cp.tile([128, GD], F32, tag="bvb", name="bvb")
            msk = cp.tile([128, 128], BF16, tag="msk", name="msk")
            nc.sync.dma_start(bq2[:], bq_d[:])
            nc.sync.dma_start(bk2[:], bk_d[:])
            nc.sync.dma_start(bvb[:], bvb_d[:])
            nc.sync.dma_start(msk[:], msk_d[:])
            wpt = []
            for p in range(2):
                t_ = cp.tile([128, D], F32R, tag=f"wp{p}", name=f"wp{p}")
                nc.scalar.dma_start(t_[:], wpT_d[p * 128:(p + 1) * 128, :])
                wpt.append(t_)

            qt = [bp_.tile([128, T], BF16, tag=f"qt{m}", name=f"qt{m}")
                  for m in range(2)]
            kt = [bp_.tile([128, T], BF16, tag=f"kt{m}", name=f"kt{m}")
                  for m in range(2)]
            ytsb = [bp_.tile([128, T], F32R, tag=f"yt{p}", name=f"yt{p}")
                    for p in range(2)]
            vt = [bp_.tile([128, 4, 2, DH], BF16, tag=f"v{t}", name=f"v{t}")
                  for t in range(TB)]

            def qk_group(dst, w, b2, m, n):
                ps = pC.tile([128, 512], F32, tag=pC.name, name="psqk")
                for k in range(KT):
                    nc.tensor.matmul(
                        ps[:],
                        w[:, k, m * 128:(m + 1) * 128],
                        xt[k][:, n * 512:(n + 1) * 512],
                        start=(k == 0), stop=(k == KT - 1),
                    )
                nc.vector.tensor_scalar_add(
                    dst[m][:, n * 512:(n + 1) * 512], ps[:], b2[:, m:m + 1],
                )

            def v_group(t):
                nc.gpsimd.memset(vt[t][:, :, 1, :], 1.0)
                ps = pC.tile([128, 512], F32, tag=pC.name, name="psv")
                for k in range(KT):
                    nc.tensor.matmul(
                        ps[:, 0:GD],
                        xt[k][:, t * 128:(t + 1) * 128],
                        wv[:, k, :],
                        start=(k == 0), stop=(k == KT - 1),
                    )
                nc.vector.tensor_add(
                    vt[t][:, :, 0, :],
                    ps[:, 0:GD].rearrange("p (h d) -> p h d", h=4),
                    bvb.rearrange("p (h d) -> p h d", h=4),
                )

            def proj_group(t, copy_eng):
                ob = op_.tile([128, 1024], F32, tag="ob", name="ob")
                for n in range(2):
                    po = pC.tile([128, 512], F32, tag=pC.name, name="pso")
                    for p in range(2):
                        nc.tensor.matmul(
                            po[:],
                            ytsb[p][:, 128 * t:128 * (t + 1)],
                            wpt[p][:, 512 * n:512 * (n + 1)],
                            start=(p == 0), stop=(p == 1),
                        )
                    if (copy_eng == "act") == (n == 0):
                        nc.scalar.copy(ob[:, 512 * n:512 * (n + 1)], po[:])
                    else:
                        nc.vector.tensor_copy(ob[:, 512 * n:512 * (n + 1)], po[:])
                eng = (nc.sync, nc.scalar)[t % 2]
                eng.dma_start(out_d[128 * t:128 * (t + 1), :], ob[:])

            def attention_seg(Ti, hp, j, fillers, every, last=False):
                h = 2 * hp + j
                ytp = pB.tile([128, 1024], F32, tag=pB.name, name="psyt")
                nblk = 8 * (Ti + 1)
                SKEW = 3       # AV trails QK/exp: the PE never
                pend = []      # waits on an exp that was just issued
                def do_av(tkb, ptsb):
                    s = max(0, 128 * tkb - 1024 * Ti)
                    for bk in range(2):
                        c0, c1 = max(s, 512 * bk), 512 * (bk + 1)
                        if c0 >= c1:
                            continue
                        nc.tensor.matmul(
                            ytp[:, c0:c1],
                            vt[tkb][:, h, :, :].rearrange("p a d -> p (a d)"),
                            ptsb[:, c0:c1],
                            start=(tkb == 0), stop=(tkb == nblk - 1),
                        )
                for tkb in range(nblk + SKEW):
                    if tkb < nblk:
                        s = max(0, 128 * tkb - 1024 * Ti)
                        pt = pA.tile([128, 1024], F32, tag=pA.name,
                                     name="pspt")
                        for bk in range(2):
                            c0, c1 = max(s, 512 * bk), 512 * (bk + 1)
                            if c0 >= c1:
                                continue
                            nc.tensor.matmul(
                                pt[:, c0:c1],
                                kt[hp][64 * j:64 * j + 64,
                                       128 * tkb:128 * (tkb + 1)],
                                qt[hp][64 * j:64 * j + 64,
                                       1024 * Ti + c0:1024 * Ti + c1],
                                start=True, stop=True,
                            )
                        ptsb = wp_.tile([128, 1024], BF16, tag="ptsb",
                                        name="ptsb", bufs=6)
                        nc.scalar.activation(
                            ptsb[:, s:1024], pt[:, s:1024],
                            mybir.ActivationFunctionType.Exp, scale=SCL,
                        )
                        if 128 * tkb >= 1024 * Ti:  # diagonal block
                            nc.gpsimd.tensor_mul(
                                ptsb[:, s:s + 128], ptsb[:, s:s + 128],
                                msk[:],
                            )
                        pend.append((tkb, ptsb))
                    if tkb >= SKEW:
                        do_av(*pend.pop(0))
                    if fillers and tkb % every == every - 1:
                        fillers.pop(0)()
                while pend:
                    do_av(*pend.pop(0))
                # free the PSUM accumulator promptly; 1/sums is computed as
                # exp(-ln(sums)) on ACT (same table set as the attention exp),
                # keeping the expensive reciprocal off the DVE
                if not last:
                    src = wp_.tile([128, 1024], F32, tag="ysb", name="ysb")
                    nc.vector.tensor_copy(src[:], ytp[:])
                else:
                    src = ytp
                rc = wp_.tile([64, 1024], F32, tag="recip", name="recip")
                for q in range(4):
                    cs = slice(256 * q, 256 * (q + 1))
                    nc.vector.reciprocal(rc[:, cs], src[64:128, cs])
                    nc.vector.tensor_mul(
                        ytsb[hp][64 * j:64 * j + 64,
                                 1024 * Ti + 256 * q:1024 * Ti + 256 * (q + 1)],
                        src[0:64, cs], rc[:, cs],
                    )

            # ---- schedule: only q/k m0 runs before attention; everything
            # else drips into attention segments as PE filler groups ----
            for n in range(4):
                qk_group(qt, wq, bq2, 0, n)
                qk_group(kt, wk, bk2, 0, n)

            f_v07 = [lambda t=t: v_group(t) for t in range(8)]
            f_qk1 = []
            for n in range(4):
                f_qk1.append(lambda n=n: qk_group(kt, wk, bk2, 1, n))
                f_qk1.append(lambda n=n: qk_group(qt, wq, bq2, 1, n))
            f_v = [lambda t=t: v_group(t) for t in range(8, 16)]
            f_p0 = [lambda t=t: proj_group(t, "act") for t in range(8)]

            # dummy matmuls whose only purpose is to keep the PE HAM-warm
            # through exp-paced stretches; each segment accumulates into one
            # scratch PSUM tile read once, chained into a debug output so
            # nothing dead-code-eliminates them
            wrm = bp_.tile([128, 1], F32, tag="wrm", name="wrm")
            def warm_seg(n):
                ps = pC.tile([128, 512], F32, tag=pC.name, name="pswrm")
                box = [0]
                def g():
                    nc.tensor.matmul(ps[:], msk[:], qt[0][:, 0:512],
                                     start=(box[0] == 0), stop=(box[0] == n - 1))
                    box[0] += 1
                def fin():
                    nc.vector.reduce_max(wrm[:], ps[:],
                                         axis=mybir.AxisListType.X)
                return [g] * (n - 1) + [lambda: (g(), fin())]

            def interleave(a, b):
                out = []
                for x, y in zip(a, b):
                    out += [x, y]
                return out

            attention_seg(0, 0, 0, f_v07, 1)
            attention_seg(0, 0, 1, f_qk1, 1)
            attention_seg(0, 1, 0, f_v, 1)
            attention_seg(0, 1, 1, warm_seg(8), 2)
            attention_seg(1, 0, 0, warm_seg(8), 2)
            attention_seg(1, 0, 1, f_p0[0:4], 3)
            attention_seg(1, 1, 0, f_p0[4:8], 3)
            attention_seg(1, 1, 1, warm_seg(8), 2, last=True)
            for t in range(8, 16):
                proj_group(t, "act")
            nc.sync.dma_start(wrm_d[:], wrm[:])

    nc.compile()
    return nc


def _shard(x, Wq, bq, Wk, bk, Wv, bv, Wp, bp):
    import ml_dtypes
    f32 = np.float32
    bf16 = ml_dtypes.bfloat16
    mask01 = np.triu(np.ones((128, 128), f32)).astype(bf16)
    in_maps = []
    for c in range(N_CORES):
        b, g = divmod(c, HPC)
        sl = slice(GD * g, GD * (g + 1))
        in_maps.append({
            "xT": np.ascontiguousarray(x[b].T).astype(bf16),
            "wqT": np.ascontiguousarray(Wq[sl, :].T).astype(bf16),
            "wkT": np.ascontiguousarray(Wk[sl, :].T).astype(bf16),
            "wvT": np.ascontiguousarray(Wv[sl, :].T).astype(bf16),
            "wpT": np.ascontiguousarray(Wp[:, sl].T, dtype=f32),
            "bq2": np.ascontiguousarray(bq[sl].reshape(2, 128).T, dtype=f32),
            "bk2": np.ascontiguousarray(bk[sl].reshape(2, 128).T, dtype=f32),
            "bvb": np.broadcast_to(bv[sl], (128, GD)).astype(f32),
            "mask01": mask01,
        })
    return in_maps


def run(inputs, trace=False):
    """Run the SPMD kernel; returns (output [B,T,D] f32, BassKernelResults)."""
    if "nc" not in _cache:
        _cache["nc"] = _build()
    nc = _cache["nc"]
    in_maps = _shard(**inputs)
    if trace:
        _install_ntff_hook()
    res = bass_utils.run_bass_kernel_spmd(
        nc, in_maps, core_ids=list(range(N_CORES)), trace=trace,
    )
    bp = np.asarray(inputs["bp"], dtype=np.float32)
    out = np.empty((B, T, D), dtype=np.float32)
    for b in range(B):
        acc = res.results[4 * b]["out"].astype(np.float32)
        for g in range(1, HPC):
            acc = acc + res.results[4 * b + g]["out"]
        out[b] = acc + bp
    return out, res


def kernel(**inputs):
    out, _ = run(inputs, trace=False)
    return out


def _install_ntff_hook():
    """antenv.axon_hooks is absent on this image; inject it so
    run_bass_kernel_spmd(trace=True) can capture NTFF profiles."""
    import sys, types
    if "antenv.axon_hooks" in sys.modules:
        return
    try:
        mod = types.ModuleType("antenv.axon_hooks")
        mod._hook = None
        mod.set_axon_ntff_profile_hook = lambda h: setattr(mod, "_hook", h)
        mod.get_axon_ntff_profile_hook = lambda: mod._hook
        sys.modules["antenv.axon_hooks"] = mod
        import antenv
        antenv.axon_hooks = mod
        from trn_agent_boot.trn_boot import _ntff_profile_via_ctypes
        mod.set_axon_ntff_profile_hook(
            _ntff_profile_via_ctypes("/opt/axon/libaxon_pjrt.so"))
    except Exception:
        pass


# revision 26
# speedup vs baseline: 1.1511x; 1.0055x over previous
"""Causal self-attention (B=2, T=2048, D=1024, H=16, Dh=64) on 8 TRN2 cores.

Sharding: core c = 4*b + g -> batch b (data parallel), head group g of 4
heads (tensor parallel on heads for Wq/Wk/Wv, column-split of the proj
input with the resulting partial-sum reduction done host-side at unshard).

Per-core dataflow (layouts chosen so no on-device transposes are needed):
  qT,kT [256, 2048] bf16 = W{q,k}_g @ x.T   (lhsT = W{q,k}_g.T from host)
  v     [t-block 128, 4 heads x (64 v | 64 ones)] bf16
  attention, transposed: PT[tk, tq] = kT_h-block.T @ qT_h (bf16), exp on
  ACT -> bf16, causal mask as post-exp 0/1 multiply on GPSIMD,
  AV: yT[d, tq] + softmax column sums free via the ones columns of v
  normalize: yT * recip(sums) -> ytsb [256, 2048] f32r (proj lhsT layout)
  proj partial: out[t, :] = ytsb.T-block @ Wp_gT  (f32r)
Host: out[b] = sum_g partial[4b+g] + bp.

Emission order interleaves projection work between attention segments to
keep the PE HAM-warm, and frees PSUM accumulators before the (expensive)
reciprocal so the vector engine never stalls the PE.
"""

import numpy as np

import concourse.bass as bass
import concourse.mybir as mybir
import concourse.tile as tile
from concourse import bacc
from concourse import bass_utils

F32 = mybir.dt.float32
F32R = mybir.dt.float32r
BF16 = mybir.dt.bfloat16

B, T, D = 2, 2048, 1024
H, DH = 16, 64
N_CORES = 8
HPC = 4            # heads per core
GD = HPC * DH      # 256 feature cols per core
KT = D // 128      # 8 k-tiles over the model dim
TB = T // 128      # 16 t-blocks of 128
SCL = 0.125        # logit scale 1/sqrt(Dh)

_cache = {}


def _build():
    nc = bacc.Bacc("TRN2", target_bir_lowering=False, debug=False,
                   num_devices=N_CORES)

    xT_d = nc.dram_tensor("xT", [D, T], BF16, kind="ExternalInput")
    wqT_d = nc.dram_tensor("wqT", [D, GD], BF16, kind="ExternalInput")
    wkT_d = nc.dram_tensor("wkT", [D, GD], BF16, kind="ExternalInput")
    wvT_d = nc.dram_tensor("wvT", [D, GD], BF16, kind="ExternalInput")
    wpT_d = nc.dram_tensor("wpT", [GD, D], F32R, kind="ExternalInput")
    bq_d = nc.dram_tensor("bq2", [128, 2], F32, kind="ExternalInput")
    bk_d = nc.dram_tensor("bk2", [128, 2], F32, kind="ExternalInput")
    bvb_d = nc.dram_tensor("bvb", [128, GD], F32, kind="ExternalInput")
    msk_d = nc.dram_tensor("mask01", [128, 128], BF16, kind="ExternalInput")
    out_d = nc.dram_tensor("out", [T, D], F32, kind="ExternalOutput")
    wrm_d = nc.dram_tensor("wrm", [128, 1], F32, kind="ExternalOutput")

    with tile.TileContext(nc) as tc:
        with (
            tc.tile_pool(name="const", bufs=1) as cp,
            tc.tile_pool(name="big", bufs=1) as bp_,
            tc.tile_pool(name="work", bufs=4) as wp_,
            tc.tile_pool(name="outp", bufs=6) as op_,
            tc.tile_pool(name="pA", bufs=2, space="PSUM") as pA,
            tc.tile_pool(name="pB", bufs=1, space="PSUM") as pB,
            tc.tile_pool(name="pC", bufs=2, space="PSUM") as pC,
        ):
            # ---- loads: wq/wk first (qk-proj is the first compute), x
            # spread over three DMA-capable queues ----
            wq = cp.tile([128, KT, GD], BF16, tag="wq", name="wq")
            wk = cp.tile([128, KT, GD], BF16, tag="wk", name="wk")
            wv = cp.tile([128, KT, GD], BF16, tag="wv", name="wv")
            nc.scalar.dma_start(wq[:], wqT_d.rearrange("(a p) m -> p a m", p=128))
            nc.sync.dma_start(wk[:], wkT_d.rearrange("(a p) m -> p a m", p=128))
            xt = []
            for k in range(KT):
                t_ = cp.tile([128, T], BF16, tag=f"xt{k}", name=f"xt{k}")
                eng = (nc.sync, nc.scalar, nc.gpsimd)[k % 3]
                eng.dma_start(t_[:], xT_d[k * 128:(k + 1) * 128, :])
                xt.append(t_)
            nc.gpsimd.dma_start(wv[:], wvT_d.rearrange("(a p) m -> p a m", p=128))
            bq2 = cp.tile([128, 2], F32, tag="bq2", name="bq2")
            bk2 = cp.tile([128, 2], F32, tag="bk2", name="bk2")
            bvb = cp.tile([128, GD], F32, tag="bvb", name="bvb")
            msk = cp.tile([128, 128], BF16, tag="msk", name="msk")
            nc.sync.dma_start(bq2[:], bq_d[:])
            nc.sync.dma_start(bk2[:], bk_d[:])
            nc.sync.dma_start(bvb[:], bvb_d[:])
            nc.sync.dma_start(msk[:], msk_d[:])
            wpt = []
            for p in range(2):
                t_ = cp.tile([128, D], F32R, tag=f"wp{p}", name=f"wp{p}")
                nc.scalar.dma_start(t_[:], wpT_d[p * 128:(p + 1) * 128, :])
                wpt.append(t_)

            qt = [bp_.tile([128, T], BF16, tag=f"qt{m}", name=f"qt{m}")
                  for m in range(2)]
            kt = [bp_.tile([128, T], BF16, tag=f"kt{m}", name=f"kt{m}")
                  for m in range(2)]
            ytsb = [bp_.tile([128, T], F32R, tag=f"yt{p}", name=f"yt{p}")
                    for p in range(2)]
            vt = [bp_.tile([128, 4, 2, DH], BF16, tag=f"v{t}", name=f"v{t}")
                  for t in range(TB)]

            def qk_group(dst, w, b2, m, n):
                ps = pC.tile([128, 512], F32, tag=pC.name, name="psqk")
                for k in range(KT):
                    nc.tensor.matmul(
                        ps[:],
                        w[:, k, m * 128:(m + 1) * 128],
                        xt[k][:, n * 512:(n + 1) * 512],
                        start=(k == 0), stop=(k == KT - 1),
                    )
                nc.vector.tensor_scalar_add(
                    dst[m][:, n * 512:(n + 1) * 512], ps[:], b2[:, m:m + 1],
                )

            def v_group(t):
                nc.gpsimd.memset(vt[t][:, :, 1, :], 1.0)
                ps = pC.tile([128, 512], F32, tag=pC.name, name="psv")
                for k in range(KT):
                    nc.tensor.matmul(
                        ps[:, 0:GD],
                        xt[k][:, t * 128:(t + 1) * 128],
                        wv[:, k, :],
                        start=(k == 0), stop=(k == KT - 1),
                    )
                nc.vector.tensor_add(
                    vt[t][:, :, 0, :],
                    ps[:, 0:GD].rearrange("p (h d) -> p h d", h=4),
                    bvb.rearrange("p (h d) -> p h d", h=4),
                )

            def proj_group(t, copy_eng):
                ob = op_.tile([128, 1024], F32, tag="ob", name="ob")
                for n in range(2):
                    po = pC.tile([128, 512], F32, tag=pC.name, name="pso")
                    for p in range(2):
                        nc.tensor.matmul(
                            po[:],
                            ytsb[p][:, 128 * t:128 * (t + 1)],
                            wpt[p][:, 512 * n:512 * (n + 1)],
                            start=(p == 0), stop=(p == 1),
                        )
                    if (copy_eng == "act") == (n == 0):
                        nc.scalar.copy(ob[:, 512 * n:512 * (n + 1)], po[:])
                    else:
                        nc.vector.tensor_copy(ob[:, 512 * n:512 * (n + 1)], po[:])
                eng = (nc.sync, nc.scalar)[t % 2]
                eng.dma_start(out_d[128 * t:128 * (t + 1), :], ob[:])

            def attention_seg(Ti, hp, j, fillers, every, last=False):
                h = 2 * hp + j
                ytp = pB.tile([128, 1024], F32, tag=pB.name, name="psyt")
                nblk = 8 * (Ti + 1)
                SKEW = 3       # AV trails QK/exp: the PE never
                pend = []      # waits on an exp that was just issued
                def do_av(tkb, ptsb):
                    s = max(0, 128 * tkb - 1024 * Ti)
                    for bk in range(2):
                        c0, c1 = max(s, 512 * bk), 512 * (bk + 1)
                        if c0 >= c1:
                            continue
                        nc.tensor.matmul(
                            ytp[:, c0:c1],
                            vt[tkb][:, h, :, :].rearrange("p a d -> p (a d)"),
                            ptsb[:, c0:c1],
                            start=(tkb == 0), stop=(tkb == nblk - 1),
                        )
                for tkb in range(nblk + SKEW):
                    if tkb < nblk:
                        s = max(0, 128 * tkb - 1024 * Ti)
                        pt = pA.tile([128, 1024], F32, tag=pA.name,
                                     name="pspt")
                        for bk in range(2):
                            c0, c1 = max(s, 512 * bk), 512 * (bk + 1)
                            if c0 >= c1:
                                continue
                            nc.tensor.matmul(
                                pt[:, c0:c1],
                                kt[hp][64 * j:64 * j + 64,
                                       128 * tkb:128 * (tkb + 1)],
                                qt[hp][64 * j:64 * j + 64,
                                       1024 * Ti + c0:1024 * Ti + c1],
                                start=True, stop=True,
                            )
                        ptsb = wp_.tile([128, 1024], BF16, tag="ptsb",
                                        name="ptsb", bufs=6)
                        nc.scalar.activation(
                            ptsb[:, s:1024], pt[:, s:1024],
                            mybir.ActivationFunctionType.Exp, scale=SCL,
                        )
                        if 128 * tkb >= 1024 * Ti:  # diagonal block
                            nc.gpsimd.tensor_mul(
                                ptsb[:, s:s + 128], ptsb[:, s:s + 128],
                                msk[:],
                            )
                        pend.append((tkb, ptsb))
                    if tkb >= SKEW:
                        do_av(*pend.pop(0))
                    if fillers and tkb % every == every - 1:
                        fillers.pop(0)()
                while pend:
                    do_av(*pend.pop(0))
                # free the PSUM accumulator promptly; 1/sums is computed as
                # exp(-ln(sums)) on ACT (same table set as the attention exp),
                # keeping the expensive reciprocal off the DVE
                if not last:
                    src = wp_.tile([128, 1024], F32, tag="ysb", name="ysb")
                    nc.vector.tensor_copy(src[:], ytp[:])
                else:
                    src = ytp
                rc = wp_.tile([64, 1024], F32, tag="recip", name="recip")
                for q in range(4):
                    cs = slice(256 * q, 256 * (q + 1))
                    nc.vector.reciprocal(rc[:, cs], src[64:128, cs])
                    nc.vector.tensor_mul(
                        ytsb[hp][64 * j:64 * j + 64,
                                 1024 * Ti + 256 * q:1024 * Ti + 256 * (q + 1)],
                        src[0:64, cs], rc[:, cs],
                    )

            # ---- schedule: only q/k m0 runs before attention; everything
            # else drips into attention segments as PE filler groups ----
            for n in range(4):
                qk_group(qt, wq, bq2, 0, n)
                qk_group(kt, wk, bk2, 0, n)

            f_v07 = [lambda t=t: v_group(t) for t in range(8)]
            f_qk1 = []
            for n in range(4):
                f_qk1.append(lambda n=n: qk_group(kt, wk, bk2, 1, n))
                f_qk1.append(lambda n=n: qk_group(qt, wq, bq2, 1, n))
            f_v = [lambda t=t: v_group(t) for t in range(8, 16)]
            f_p0 = [lambda t=t: proj_group(t, "act") for t in range(8)]

            # dummy matmuls whose only purpose is to keep the PE HAM-warm
            # through exp-paced stretches; each segment accumulates into one
            # scratch PSUM tile read once, chained into a debug output so
            # nothing dead-code-eliminates them
            wrm = bp_.tile([128, 1], F32, tag="wrm", name="wrm")
            def warm_seg(n):
                ps = pC.tile([128, 512], F32, tag=pC.name, name="pswrm")
                box = [0]
                def g():
                    nc.tensor.matmul(ps[:], msk[:], qt[0][:, 0:512],
                                     start=(box[0] == 0), stop=(box[0] == n - 1))
                    box[0] += 1
                def fin():
                    nc.vector.reduce_max(wrm[:], ps[:],
                                         axis=mybir.AxisListType.X)
                return [g] * (n - 1) + [lambda: (g(), fin())]

            def interleave(a, b):
                out = []
                for x, y in zip(a, b):
                    out += [x, y]
                return out

            attention_seg(0, 0, 0, f_v07, 1)
            attention_seg(0, 0, 1, f_qk1, 1)
            attention_seg(0, 1, 0, warm_seg(8), 2)
            attention_seg(0, 1, 1, warm_seg(8), 2)
            attention_seg(1, 0, 0, f_v, 2)
            attention_seg(1, 0, 1, f_p0[0:4], 3)
            attention_seg(1, 1, 0, f_p0[4:8], 3)
            attention_seg(1, 1, 1, warm_seg(8), 2, last=True)
            for t in range(8, 16):
                proj_group(t, "act")
            nc.sync.dma_start(wrm_d[:], wrm[:])

    nc.compile()
    return nc


def _shard(x, Wq, bq, Wk, bk, Wv, bv, Wp, bp):
    import ml_dtypes
    f32 = np.float32
    bf16 = ml_dtypes.bfloat16
    mask01 = np.triu(np.ones((128, 128), f32)).astype(bf16)
    in_maps = []
    for c in range(N_CORES):
        b, g = divmod(c, HPC)
        sl = slice(GD * g, GD * (g + 1))
        in_maps.append({
            "xT": np.ascontiguousarray(x[b].T).astype(bf16),
            "wqT": np.ascontiguousarray(Wq[sl, :].T).astype(bf16),
            "wkT": np.ascontiguousarray(Wk[sl, :].T).astype(bf16),
            "wvT": np.ascontiguousarray(Wv[sl, :].T).astype(bf16),
            "wpT": np.ascontiguousarray(Wp[:, sl].T, dtype=f32),
            "bq2": np.ascontiguousarray(bq[sl].reshape(2, 128).T, dtype=f32),
            "bk2": np.ascontiguousarray(bk[sl].reshape(2, 128).T, dtype=f32),
            "bvb": np.broadcast_to(bv[sl], (128, GD)).astype(f32),
            "mask01": mask01,
        })
    return in_maps


def run(inputs, trace=False):
    """Run the SPMD kernel; returns (output [B,T,D] f32, BassKernelResults)."""
    if "nc" not in _cache:
        _cache["nc"] = _build()
    nc = _cache["nc"]
    in_maps = _shard(**inputs)
    if trace:
        _install_ntff_hook()
    res = bass_utils.run_bass_kernel_spmd(
        nc, in_maps, core_ids=list(range(N_CORES)), trace=trace,
    )
    bp = np.asarray(inputs["bp"], dtype=np.float32)
    out = np.empty((B, T, D), dtype=np.float32)
    for b in range(B):
        acc = res.results[4 * b]["out"].astype(np.float32)
        for g in range(1, HPC):
            acc = acc + res.results[4 * b + g]["out"]
        out[b] = acc + bp
    return out, res


def kernel(**inputs):
    out, _ = run(inputs, trace=False)
    return out


def _install_ntff_hook():
    """antenv.axon_hooks is absent on this image; inject it so
    run_bass_kernel_spmd(trace=True) can capture NTFF profiles."""
    import sys, types
    if "antenv.axon_hooks" in sys.modules:
        return
    try:
        mod = types.ModuleType("antenv.axon_hooks")
        mod._hook = None
        mod.set_axon_ntff_profile_hook = lambda h: setattr(mod, "_hook", h)
        mod.get_axon_ntff_profile_hook = lambda: mod._hook
        sys.modules["antenv.axon_hooks"] = mod
        import antenv
        antenv.axon_hooks = mod
        from trn_agent_boot.trn_boot import _ntff_profile_via_ctypes
        mod.set_axon_ntff_profile_hook(
            _ntff_profile_via_ctypes("/opt/axon/libaxon_pjrt.so"))
    except Exception:
        pass


# revision 27
# speedup vs baseline: 1.1896x; 1.0334x over previous
"""Causal self-attention (B=2, T=2048, D=1024, H=16, Dh=64) on 8 TRN2 cores.

Sharding: core c = 4*b + g -> batch b (data parallel), head group g of 4
heads (tensor parallel on heads for Wq/Wk/Wv, column-split of the proj
input with the resulting partial-sum reduction done host-side at unshard).

Per-core dataflow (layouts chosen so no on-device transposes are needed):
  qT,kT [256, 2048] bf16 = W{q,k}_g @ x.T   (lhsT = W{q,k}_g.T from host)
  v     [t-block 128, 4 heads x (64 v | 64 ones)] bf16
  attention, transposed: PT[tk, tq] = kT_h-block.T @ qT_h (bf16), exp on
  ACT -> bf16, causal mask as post-exp 0/1 multiply on GPSIMD,
  AV: yT[d, tq] + softmax column sums free via the ones columns of v
  normalize: yT * recip(sums) -> ytsb [256, 2048] f32r (proj lhsT layout)
  proj partial: out[t, :] = ytsb.T-block @ Wp_gT  (f32r)
Host: out[b] = sum_g partial[4b+g] + bp.

Emission order interleaves projection work between attention segments to
keep the PE HAM-warm, and frees PSUM accumulators before the (expensive)
reciprocal so the vector engine never stalls the PE.
"""

import numpy as np

import concourse.bass as bass
import concourse.mybir as mybir
import concourse.tile as tile
from concourse import bacc
from concourse import bass_utils

F32 = mybir.dt.float32
F32R = mybir.dt.float32r
BF16 = mybir.dt.bfloat16

B, T, D = 2, 2048, 1024
H, DH = 16, 64
N_CORES = 8
HPC = 4            # heads per core
GD = HPC * DH      # 256 feature cols per core
KT = D // 128      # 8 k-tiles over the model dim
TB = T // 128      # 16 t-blocks of 128
SCL = 0.125        # logit scale 1/sqrt(Dh)

_cache = {}


def _build():
    nc = bacc.Bacc("TRN2", target_bir_lowering=False, debug=False,
                   num_devices=N_CORES)

    xT_d = nc.dram_tensor("xT", [D, T], BF16, kind="ExternalInput")
    wqT_d = nc.dram_tensor("wqT", [D, GD], BF16, kind="ExternalInput")
    wkT_d = nc.dram_tensor("wkT", [D, GD], BF16, kind="ExternalInput")
    wvT_d = nc.dram_tensor("wvT", [D, GD], BF16, kind="ExternalInput")
    wpT_d = nc.dram_tensor("wpT", [GD, D], F32R, kind="ExternalInput")
    bq_d = nc.dram_tensor("bq2", [128, 2], F32, kind="ExternalInput")
    bk_d = nc.dram_tensor("bk2", [128, 2], F32, kind="ExternalInput")
    bvb_d = nc.dram_tensor("bvb", [128, GD], F32, kind="ExternalInput")
    msk_d = nc.dram_tensor("mask01", [128, 128], BF16, kind="ExternalInput")
    out_d = nc.dram_tensor("out", [T, D], F32, kind="ExternalOutput")
    wrm_d = nc.dram_tensor("wrm", [128, 1], F32, kind="ExternalOutput")

    with tile.TileContext(nc) as tc:
        with (
            tc.tile_pool(name="const", bufs=1) as cp,
            tc.tile_pool(name="big", bufs=1) as bp_,
            tc.tile_pool(name="work", bufs=4) as wp_,
            tc.tile_pool(name="outp", bufs=6) as op_,
            tc.tile_pool(name="pA", bufs=2, space="PSUM") as pA,
            tc.tile_pool(name="pB", bufs=1, space="PSUM") as pB,
            tc.tile_pool(name="pC", bufs=2, space="PSUM") as pC,
        ):
            # ---- loads: wq/wk first (qk-proj is the first compute), x
            # spread over three DMA-capable queues ----
            wq = cp.tile([128, KT, GD], BF16, tag="wq", name="wq")
            wk = cp.tile([128, KT, GD], BF16, tag="wk", name="wk")
            wv = cp.tile([128, KT, GD], BF16, tag="wv", name="wv")
            nc.scalar.dma_start(wq[:], wqT_d.rearrange("(a p) m -> p a m", p=128))
            nc.sync.dma_start(wk[:], wkT_d.rearrange("(a p) m -> p a m", p=128))
            xt = []
            for k in range(KT):
                t_ = cp.tile([128, T], BF16, tag=f"xt{k}", name=f"xt{k}")
                eng = (nc.sync, nc.scalar, nc.gpsimd)[k % 3]
                eng.dma_start(t_[:], xT_d[k * 128:(k + 1) * 128, :])
                xt.append(t_)
            nc.gpsimd.dma_start(wv[:], wvT_d.rearrange("(a p) m -> p a m", p=128))
            bq2 = cp.tile([128, 2], F32, tag="bq2", name="bq2")
            bk2 = cp.tile([128, 2], F32, tag="bk2", name="bk2")
            bvb = cp.tile([128, GD], F32, tag="bvb", name="bvb")
            msk = cp.tile([128, 128], BF16, tag="msk", name="msk")
            nc.sync.dma_start(bq2[:], bq_d[:])
            nc.sync.dma_start(bk2[:], bk_d[:])
            nc.sync.dma_start(bvb[:], bvb_d[:])
            nc.sync.dma_start(msk[:], msk_d[:])
            wpt = []
            for p in range(2):
                t_ = cp.tile([128, D], F32R, tag=f"wp{p}", name=f"wp{p}")
                nc.scalar.dma_start(t_[:], wpT_d[p * 128:(p + 1) * 128, :])
                wpt.append(t_)

            qt = [bp_.tile([128, T], BF16, tag=f"qt{m}", name=f"qt{m}")
                  for m in range(2)]
            kt = [bp_.tile([128, T], BF16, tag=f"kt{m}", name=f"kt{m}")
                  for m in range(2)]
            ytsb = [bp_.tile([128, T], F32R, tag=f"yt{p}", name=f"yt{p}")
                    for p in range(2)]
            vt = [bp_.tile([128, 4, 2, DH], BF16, tag=f"v{t}", name=f"v{t}")
                  for t in range(TB)]

            def qk_group(dst, w, b2, m, n):
                ps = pC.tile([128, 512], F32, tag=pC.name, name="psqk")
                for k in range(KT):
                    nc.tensor.matmul(
                        ps[:],
                        w[:, k, m * 128:(m + 1) * 128],
                        xt[k][:, n * 512:(n + 1) * 512],
                        start=(k == 0), stop=(k == KT - 1),
                    )
                nc.vector.tensor_scalar_add(
                    dst[m][:, n * 512:(n + 1) * 512], ps[:], b2[:, m:m + 1],
                )

            def v_group(t):
                nc.gpsimd.memset(vt[t][:, :, 1, :], 1.0)
                ps = pC.tile([128, 512], F32, tag=pC.name, name="psv")
                for k in range(KT):
                    nc.tensor.matmul(
                        ps[:, 0:GD],
                        xt[k][:, t * 128:(t + 1) * 128],
                        wv[:, k, :],
                        start=(k == 0), stop=(k == KT - 1),
                    )
                nc.vector.tensor_add(
                    vt[t][:, :, 0, :],
                    ps[:, 0:GD].rearrange("p (h d) -> p h d", h=4),
                    bvb.rearrange("p (h d) -> p h d", h=4),
                )

            def proj_group(t, copy_eng):
                ob = op_.tile([128, 1024], F32, tag="ob", name="ob")
                for n in range(2):
                    po = pC.tile([128, 512], F32, tag=pC.name, name="pso")
                    for p in range(2):
                        nc.tensor.matmul(
                            po[:],
                            ytsb[p][:, 128 * t:128 * (t + 1)],
                            wpt[p][:, 512 * n:512 * (n + 1)],
                            start=(p == 0), stop=(p == 1),
                        )
                    if (copy_eng == "act") == (n == 0):
                        nc.scalar.copy(ob[:, 512 * n:512 * (n + 1)], po[:])
                    else:
                        nc.vector.tensor_copy(ob[:, 512 * n:512 * (n + 1)], po[:])
                eng = (nc.sync, nc.scalar)[t % 2]
                eng.dma_start(out_d[128 * t:128 * (t + 1), :], ob[:])

            def attention_seg(Ti, hp, j, fillers, every, last=False):
                h = 2 * hp + j
                ytp = pB.tile([128, 1024], F32, tag=pB.name, name="psyt")
                nblk = 8 * (Ti + 1)
                SKEW = 3       # AV trails QK/exp: the PE never
                pend = []      # waits on an exp that was just issued
                def do_av(tkb, ptsb):
                    s = max(0, 128 * tkb - 1024 * Ti)
                    for bk in range(2):
                        c0, c1 = max(s, 512 * bk), 512 * (bk + 1)
                        if c0 >= c1:
                            continue
                        nc.tensor.matmul(
                            ytp[:, c0:c1],
                            vt[tkb][:, h, :, :].rearrange("p a d -> p (a d)"),
                            ptsb[:, c0:c1],
                            start=(tkb == 0), stop=(tkb == nblk - 1),
                        )
                for tkb in range(nblk + SKEW):
                    if tkb < nblk:
                        s = max(0, 128 * tkb - 1024 * Ti)
                        pt = pA.tile([128, 1024], F32, tag=pA.name,
                                     name="pspt")
                        for bk in range(2):
                            c0, c1 = max(s, 512 * bk), 512 * (bk + 1)
                            if c0 >= c1:
                                continue
                            nc.tensor.matmul(
                                pt[:, c0:c1],
                                kt[hp][64 * j:64 * j + 64,
                                       128 * tkb:128 * (tkb + 1)],
                                qt[hp][64 * j:64 * j + 64,
                                       1024 * Ti + c0:1024 * Ti + c1],
                                start=True, stop=True,
                            )
                        ptsb = wp_.tile([128, 1024], BF16, tag="ptsb",
                                        name="ptsb", bufs=6)
                        nc.scalar.activation(
                            ptsb[:, s:1024], pt[:, s:1024],
                            mybir.ActivationFunctionType.Exp, scale=SCL,
                        )
                        if 128 * tkb >= 1024 * Ti:  # diagonal block
                            nc.gpsimd.tensor_mul(
                                ptsb[:, s:s + 128], ptsb[:, s:s + 128],
                                msk[:],
                            )
                        pend.append((tkb, ptsb))
                    if tkb >= SKEW:
                        do_av(*pend.pop(0))
                    if fillers and tkb % every == every - 1:
                        fillers.pop(0)()
                while pend:
                    do_av(*pend.pop(0))
                # free the PSUM accumulator promptly; 1/sums is computed as
                # exp(-ln(sums)) on ACT (same table set as the attention exp),
                # keeping the expensive reciprocal off the DVE
                if not last:
                    src = wp_.tile([128, 1024], F32, tag="ysb", name="ysb")
                    nc.vector.tensor_copy(src[:], ytp[:])
                else:
                    src = ytp
                rc = wp_.tile([64, 1024], F32, tag="recip", name="recip")
                for q in range(4):
                    cs = slice(256 * q, 256 * (q + 1))
                    nc.vector.reciprocal(rc[:, cs], src[64:128, cs])
                    nc.vector.tensor_mul(
                        ytsb[hp][64 * j:64 * j + 64,
                                 1024 * Ti + 256 * q:1024 * Ti + 256 * (q + 1)],
                        src[0:64, cs], rc[:, cs],
                    )

            # ---- schedule: only q/k m0 runs before attention; everything
            # else drips into attention segments as PE filler groups ----
            for n in range(4):
                qk_group(qt, wq, bq2, 0, n)
                qk_group(kt, wk, bk2, 0, n)

            f_v07 = [lambda t=t: v_group(t) for t in range(8)]
            f_qk1a, f_qk1b = [], []
            for n in range(4):
                dst = f_qk1a if n < 2 else f_qk1b
                dst.append(lambda n=n: qk_group(kt, wk, bk2, 1, n))
                dst.append(lambda n=n: qk_group(qt, wq, bq2, 1, n))
            f_v = [lambda t=t: v_group(t) for t in range(8, 16)]
            f_p0 = [lambda t=t: proj_group(t, "act") for t in range(8)]

            # dummy matmuls whose only purpose is to keep the PE HAM-warm
            # through exp-paced stretches; each segment accumulates into one
            # scratch PSUM tile read once, chained into a debug output so
            # nothing dead-code-eliminates them
            wrm = bp_.tile([128, 1], F32, tag="wrm", name="wrm")
            def warm_seg(n):
                ps = pC.tile([128, 512], F32, tag=pC.name, name="pswrm")
                box = [0]
                def g():
                    nc.tensor.matmul(ps[:], msk[:], qt[0][:, 0:512],
                                     start=(box[0] == 0), stop=(box[0] == n - 1))
                    box[0] += 1
                def fin():
                    nc.vector.reduce_max(wrm[:], ps[:],
                                         axis=mybir.AxisListType.X)
                return [g] * (n - 1) + [lambda: (g(), fin())]

            def interleave(a, b):
                out = []
                for x, y in zip(a, b):
                    out += [x, y]
                return out

            attention_seg(0, 0, 0, f_v07, 1)
            attention_seg(0, 0, 1, f_qk1a, 2)
            attention_seg(0, 1, 0, f_qk1b, 2)
            attention_seg(0, 1, 1, warm_seg(8), 2)
            attention_seg(1, 0, 0, f_v, 2)
            attention_seg(1, 0, 1, f_p0[0:4], 3)
            attention_seg(1, 1, 0, f_p0[4:8], 3)
            attention_seg(1, 1, 1, warm_seg(8), 2, last=True)
            for t in range(8, 16):
                proj_group(t, "act")
            nc.sync.dma_start(wrm_d[:], wrm[:])

    nc.compile()
    return nc


def _shard(x, Wq, bq, Wk, bk, Wv, bv, Wp, bp):
    import ml_dtypes
    f32 = np.float32
    bf16 = ml_dtypes.bfloat16
    mask01 = np.triu(np.ones((128, 128), f32)).astype(bf16)
    in_maps = []
    for c in range(N_CORES):
        b, g = divmod(c, HPC)
        sl = slice(GD * g, GD * (g + 1))
        in_maps.append({
            "xT": np.ascontiguousarray(x[b].T).astype(bf16),
            "wqT": np.ascontiguousarray(Wq[sl, :].T).astype(bf16),
            "wkT": np.ascontiguousarray(Wk[sl, :].T).astype(bf16),
            "wvT": np.ascontiguousarray(Wv[sl, :].T).astype(bf16),
            "wpT": np.ascontiguousarray(Wp[:, sl].T, dtype=f32),
            "bq2": np.ascontiguousarray(bq[sl].reshape(2, 128).T, dtype=f32),
            "bk2": np.ascontiguousarray(bk[sl].reshape(2, 128).T, dtype=f32),
            "bvb": np.broadcast_to(bv[sl], (128, GD)).astype(f32),
            "mask01": mask01,
        })
    return in_maps


def run(inputs, trace=False):
    """Run the SPMD kernel; returns (output [B,T,D] f32, BassKernelResults)."""
    if "nc" not in _cache:
        _cache["nc"] = _build()
    nc = _cache["nc"]
    in_maps = _shard(**inputs)
    if trace:
        _install_ntff_hook()
    res = bass_utils.run_bass_kernel_spmd(
        nc, in_maps, core_ids=list(range(N_CORES)), trace=trace,
    )
    bp = np.asarray(inputs["bp"], dtype=np.float32)
    out = np.empty((B, T, D), dtype=np.float32)
    for b in range(B):
        acc = res.results[4 * b]["out"].astype(np.float32)
        for g in range(1, HPC):
            acc = acc + res.results[4 * b + g]["out"]
        out[b] = acc + bp
    return out, res


def kernel(**inputs):
    out, _ = run(inputs, trace=False)
    return out


def _install_ntff_hook():
    """antenv.axon_hooks is absent on this image; inject it so
    run_bass_kernel_spmd(trace=True) can capture NTFF profiles."""
    import sys, types
    if "antenv.axon_hooks" in sys.modules:
        return
    try:
        mod = types.ModuleType("antenv.axon_hooks")
        mod._hook = None
        mod.set_axon_ntff_profile_hook = lambda h: setattr(mod, "_hook", h)
        mod.get_axon_ntff_profile_hook = lambda: mod._hook
        sys.modules["antenv.axon_hooks"] = mod
        import antenv
        antenv.axon_hooks = mod
        from trn_agent_boot.trn_boot import _ntff_profile_via_ctypes
        mod.set_axon_ntff_profile_hook(
            _ntff_profile_via_ctypes("/opt/axon/libaxon_pjrt.so"))
    except Exception:
        pass
